# revision 1
# baseline (speedup 1.0000x reference)
"""Trainium2 Bass kernel for nn_NodeBlock (gnn_message_passing).

reference semantics:
    agg_mesh  = segment_sum(edge_attr, receivers, N)
    agg_world = segment_sum(edge_world_attr, receivers_world, N)
    h = concat([node_attr, agg_mesh, agg_world], -1)   # [N, 3D]
    h = relu(h @ W1 + b1) @ W2 + b2
    out = layernorm(h) * gamma + beta

Strategy (8 cores, sharded by node owner):
  - nodes split contiguously: core c owns [c*NPC, (c+1)*NPC)
  - edges partitioned by receiver owner, sorted by receiver, packed on host
    into per-node-tile chunk-aligned buffers (rows padded; pad rows carry a
    sentinel local-index so they never match the one-hot compare)
  - on device, per 128-node tile: scatter = sum_chunks edge_chunk^T @ P where
    P[e, j] = (iota[j] == r_local[e]) built with a single tensor_scalar
    is_equal; aggregates accumulate transposed [D, node] in PSUM, feeding the
    MLP matmuls directly (no on-device transposes; node_attr comes in
    pre-transposed from the host).
"""

import os

import numpy as np

LN_EPS = 1e-5
NC_CORES = 8
P = 128


def _build_program(T, CHm, CHw, D):
    import concourse.bacc as bacc
    import concourse.tile as tile
    from concourse import mybir

    f32 = mybir.dt.float32
    nc = bacc.Bacc("TRN2")

    em = nc.dram_tensor("em", [T * CHm * P, D], f32, kind="ExternalInput")
    ew = nc.dram_tensor("ew", [T * CHw * P, D], f32, kind="ExternalInput")
    rm = nc.dram_tensor("rm", [T, P, CHm], f32, kind="ExternalInput")
    rw = nc.dram_tensor("rw", [T, P, CHw], f32, kind="ExternalInput")
    ndT = nc.dram_tensor("ndT", [P, T * P], f32, kind="ExternalInput")
    w1 = nc.dram_tensor("w1", [3 * P, D], f32, kind="ExternalInput")
    w2 = nc.dram_tensor("w2", [P, D], f32, kind="ExternalInput")
    b1 = nc.dram_tensor("b1", [P, 1], f32, kind="ExternalInput")
    gb = nc.dram_tensor("gb", [P, D], f32, kind="ExternalInput")
    bb = nc.dram_tensor("bb", [P, D], f32, kind="ExternalInput")
    b2b = nc.dram_tensor("b2b", [P, D], f32, kind="ExternalInput")
    iot = nc.dram_tensor("iot", [P, P], f32, kind="ExternalInput")
    outd = nc.dram_tensor("out", [T * P, D], f32, kind="ExternalOutput")

    with tile.TileContext(nc) as tc:
        with (
            tc.tile_pool(name="consts", bufs=1) as consts,
            tc.tile_pool(name="edges", bufs=3) as edges,
            tc.tile_pool(name="nodes", bufs=3) as nodes,
            tc.tile_pool(name="ridx", bufs=3) as ridx,
            tc.tile_pool(name="ponehot", bufs=4) as ponehot,
            tc.tile_pool(name="aggs", bufs=3) as aggs,
            tc.tile_pool(name="work", bufs=3) as work,
            tc.tile_pool(name="small", bufs=4) as small,
            tc.tile_pool(name="psum", bufs=2, space="PSUM") as psum,
        ):
            w1s = consts.tile([P, 3, D], f32)
            nc.sync.dma_start(out=w1s, in_=w1[:, :].rearrange("(j p) d -> p j d", p=P))
            w2s = consts.tile([P, D], f32)
            nc.sync.dma_start(out=w2s, in_=w2[:, :])
            b1s = consts.tile([P, 1], f32)
            nc.sync.dma_start(out=b1s, in_=b1[:, :])
            gbs = consts.tile([P, D], f32)
            nc.sync.dma_start(out=gbs, in_=gb[:, :])
            bbs = consts.tile([P, D], f32)
            nc.sync.dma_start(out=bbs, in_=bb[:, :])
            b2s = consts.tile([P, D], f32)
            nc.sync.dma_start(out=b2s, in_=b2b[:, :])
            iots = consts.tile([P, P], f32)
            nc.sync.dma_start(out=iots, in_=iot[:, :])

            for t in range(T):
                # ---- loads ----
                nds = nodes.tile([P, P], f32, tag="nds")
                nc.sync.dma_start(out=nds, in_=ndT[:, t * P : (t + 1) * P])
                emt = edges.tile([P, CHm, D], f32, tag="emt")
                nc.sync.dma_start(
                    out=emt,
                    in_=em[t * CHm * P : (t + 1) * CHm * P, :].rearrange(
                        "(c p) d -> p c d", p=P
                    ),
                )
                ewt = edges.tile([P, CHw, D], f32, tag="ewt")
                nc.sync.dma_start(
                    out=ewt,
                    in_=ew[t * CHw * P : (t + 1) * CHw * P, :].rearrange(
                        "(c p) d -> p c d", p=P
                    ),
                )
                rmt = ridx.tile([P, CHm], f32, tag="rmt")
                nc.sync.dma_start(out=rmt, in_=rm[t, :, :])
                rwt = ridx.tile([P, CHw], f32, tag="rwt")
                nc.sync.dma_start(out=rwt, in_=rw[t, :, :])

                # ---- scatter: agg^T[d, j] = sum_e edge[e, d] * P[e, j] ----
                aggm_ps = psum.tile([P, P], f32, tag="aggm")
                for c in range(CHm):
                    pm = ponehot.tile([P, P], f32, tag="pm")
                    nc.any.tensor_scalar(
                        out=pm,
                        in0=iots,
                        scalar1=rmt[:, c : c + 1],
                        scalar2=None,
                        op0=mybir.AluOpType.is_equal,
                    )
                    nc.tensor.matmul(
                        out=aggm_ps,
                        lhsT=emt[:, c, :],
                        rhs=pm,
                        start=(c == 0),
                        stop=(c == CHm - 1),
                    )
                aggw_ps = psum.tile([P, P], f32, tag="aggw")
                for c in range(CHw):
                    pw = ponehot.tile([P, P], f32, tag="pw")
                    nc.any.tensor_scalar(
                        out=pw,
                        in0=iots,
                        scalar1=rwt[:, c : c + 1],
                        scalar2=None,
                        op0=mybir.AluOpType.is_equal,
                    )
                    nc.tensor.matmul(
                        out=aggw_ps,
                        lhsT=ewt[:, c, :],
                        rhs=pw,
                        start=(c == 0),
                        stop=(c == CHw - 1),
                    )
                aggms = aggs.tile([P, P], f32, tag="aggms")
                nc.any.tensor_copy(out=aggms, in_=aggm_ps)
                aggws = aggs.tile([P, P], f32, tag="aggws")
                nc.any.tensor_copy(out=aggws, in_=aggw_ps)

                # ---- MLP: y^T[dout, n] = sum_k W1[k, dout] * h^T[k, n] ----
                y_ps = psum.tile([P, P], f32, tag="y")
                nc.tensor.matmul(
                    out=y_ps, lhsT=w1s[:, 0, :], rhs=nds, start=True, stop=False
                )
                nc.tensor.matmul(
                    out=y_ps, lhsT=w1s[:, 1, :], rhs=aggms, start=False, stop=False
                )
                nc.tensor.matmul(
                    out=y_ps, lhsT=w1s[:, 2, :], rhs=aggws, start=False, stop=True
                )
                yr = work.tile([P, P], f32, tag="yr")
                nc.scalar.activation(
                    out=yr,
                    in_=y_ps,
                    func=mybir.ActivationFunctionType.Relu,
                    bias=b1s[:, :],
                    scale=1.0,
                )
                # z[n, dz] = sum_j yr[j, n] * W2[j, dz]
                z_ps = psum.tile([P, P], f32, tag="z")
                nc.tensor.matmul(out=z_ps, lhsT=yr, rhs=w2s, start=True, stop=True)

                # ---- layernorm over free dim (+b2, *gamma, +beta) ----
                zs = work.tile([P, P], f32, tag="zs")
                musum = small.tile([P, 1], f32, tag="musum")
                nc.vector.scalar_tensor_tensor(
                    out=zs,
                    in0=z_ps,
                    scalar=1.0,
                    in1=b2s,
                    op0=mybir.AluOpType.mult,
                    op1=mybir.AluOpType.add,
                    accum_out=musum,
                )
                negmu = small.tile([P, 1], f32, tag="negmu")
                nc.scalar.mul(out=negmu, in_=musum, mul=-1.0 / D)
                zc = work.tile([P, P], f32, tag="zc")
                nc.any.tensor_scalar(
                    out=zc,
                    in0=zs,
                    scalar1=negmu,
                    scalar2=None,
                    op0=mybir.AluOpType.add,
                )
                sq = work.tile([P, P], f32, tag="sq")
                sqsum = small.tile([P, 1], f32, tag="sqsum")
                nc.scalar.activation(
                    out=sq,
                    in_=zc,
                    func=mybir.ActivationFunctionType.Square,
                    accum_out=sqsum,
                )
                veps = small.tile([P, 1], f32, tag="veps")
                nc.vector.tensor_scalar(
                    out=veps,
                    in0=sqsum,
                    scalar1=1.0 / D,
                    scalar2=LN_EPS,
                    op0=mybir.AluOpType.mult,
                    op1=mybir.AluOpType.add,
                )
                rvi = small.tile([P, 1], f32, tag="rvi")
                nc.vector.reciprocal(out=rvi, in_=veps)
                rstd = small.tile([P, 1], f32, tag="rstd")
                nc.scalar.sqrt(out=rstd, in_=rvi)
                t2 = work.tile([P, P], f32, tag="t2")
                nc.vector.scalar_tensor_tensor(
                    out=t2,
                    in0=zc,
                    scalar=rstd,
                    in1=gbs,
                    op0=mybir.AluOpType.mult,
                    op1=mybir.AluOpType.mult,
                )
                outt = work.tile([P, P], f32, tag="outt")
                nc.any.tensor_tensor(
                    out=outt, in0=t2, in1=bbs, op=mybir.AluOpType.add
                )
                nc.sync.dma_start(out=outd[t * P : (t + 1) * P, :], in_=outt)

    nc.finalize()
    return nc


def _pack_edges(edge_attr, recv, NPC, T, n_cores):
    """Sort edges by receiver, partition by owner core, pack into per-tile
    chunk-aligned buffers. Returns (em_pad [n_cores, T*CH*P, D],
    rl_pad [n_cores, T, P, CH], CH)."""
    D = edge_attr.shape[1]
    order = np.argsort(recv, kind="stable")
    r_sorted = recv[order]
    c_ids = r_sorted // NPC
    loc = r_sorted - c_ids * NPC
    t_ids = loc // P
    rl = (loc - t_ids * P).astype(np.float32)
    g_ids = c_ids * T + t_ids
    cnt = np.bincount(g_ids, minlength=n_cores * T)
    CH = max(1, int(np.ceil(cnt.max() / P)))
    starts = np.cumsum(cnt) - cnt
    pos = np.arange(len(r_sorted)) - starts[g_ids]
    dest = g_ids * (CH * P) + pos

    em_pad = np.zeros((n_cores * T * CH * P, D), np.float32)
    em_pad[dest] = edge_attr[order]
    rl_pad = np.full(n_cores * T * CH * P, 300.0, np.float32)
    rl_pad[dest] = rl
    em_pad = em_pad.reshape(n_cores, T * CH * P, D)
    # [T, CH, P] -> [T, P, CH] so the device DMA reads contiguous rows
    rl_pad = np.ascontiguousarray(
        rl_pad.reshape(n_cores, T, CH, P).transpose(0, 1, 3, 2)
    )
    return em_pad, rl_pad, CH


def kernel(**inputs):
    from concourse.bass_utils import run_bass_kernel_spmd

    node_attr = np.asarray(inputs["node_attr"], np.float32)
    edge_attr = np.asarray(inputs["edge_attr"], np.float32)
    edge_world_attr = np.asarray(inputs["edge_world_attr"], np.float32)
    recv = np.asarray(inputs["receivers"]).astype(np.int64)
    recv_w = np.asarray(inputs["receivers_world"]).astype(np.int64)
    W1 = np.asarray(inputs["W1"], np.float32)
    b1 = np.asarray(inputs["b1"], np.float32)
    W2 = np.asarray(inputs["W2"], np.float32)
    b2 = np.asarray(inputs["b2"], np.float32)
    gamma = np.asarray(inputs["gamma"], np.float32)
    beta = np.asarray(inputs["beta"], np.float32)

    N, D = node_attr.shape
    assert D == P and N % NC_CORES == 0
    NPC = N // NC_CORES
    T = (NPC + P - 1) // P
    NPAD = T * P

    em_pad, rm_pad, CHm = _pack_edges(edge_attr, recv, NPC, T, NC_CORES)
    ew_pad, rw_pad, CHw = _pack_edges(edge_world_attr, recv_w, NPC, T, NC_CORES)

    # node_attr transposed per core: [P, T*P]
    ndT = np.zeros((NC_CORES, P, NPAD), np.float32)
    for c in range(NC_CORES):
        ndT[c, :, :NPC] = node_attr[c * NPC : (c + 1) * NPC].T

    iota = np.tile(np.arange(P, dtype=np.float32), (P, 1))
    gb = np.tile(gamma, (P, 1)).astype(np.float32)
    bb = np.tile(beta, (P, 1)).astype(np.float32)
    b2b = np.tile(b2, (P, 1)).astype(np.float32)
    b1c = np.ascontiguousarray(b1.reshape(P, 1))

    nc = _build_program(T, CHm, CHw, D)

    in_maps = []
    for c in range(NC_CORES):
        in_maps.append(
            {
                "em": em_pad[c],
                "ew": ew_pad[c],
                "rm": rm_pad[c],
                "rw": rw_pad[c],
                "ndT": ndT[c],
                "w1": W1,
                "w2": W2,
                "b1": b1c,
                "gb": gb,
                "bb": bb,
                "b2b": b2b,
                "iot": iota,
            }
        )

    prof_dir = os.environ.get("KERNEL_PROFILE_DIR")
    trace = False
    if prof_dir:
        try:
            _install_profile_hook()
            trace = True
        except Exception as e:  # profiling is best-effort
            print(f"profile hook unavailable: {e}")

    res = run_bass_kernel_spmd(
        nc,
        in_maps,
        core_ids=list(range(NC_CORES)),
        trace=trace,
        tmpdir=prof_dir if trace else None,
    )
    if trace:
        print(f"HW exec time: {res.exec_time_ns} ns")

    out = np.concatenate(
        [res.results[c]["out"][:NPC] for c in range(NC_CORES)], axis=0
    )
    return out


def _install_profile_hook():
    """Register the axon NTFF profile hook (the boot path skips it when
    antenv.axon_hooks is absent) and neuter the artifact upload."""
    import contextlib
    import ctypes
    import sys
    import types

    lib = ctypes.CDLL("/opt/axon/libaxon_pjrt.so")
    lib.axon_start_nrt_profile.argtypes = [
        ctypes.POINTER(ctypes.c_int64),
        ctypes.c_size_t,
    ]
    lib.axon_start_nrt_profile.restype = ctypes.c_int64
    lib.axon_stop_nrt_profile.argtypes = [ctypes.c_char_p]
    lib.axon_stop_nrt_profile.restype = ctypes.c_int64

    @contextlib.contextmanager
    def _hook(output_dir, device_ids):
        import jax

        jax.devices()
        if device_ids:
            ids = (ctypes.c_int64 * len(device_ids))(*device_ids)
            rc = lib.axon_start_nrt_profile(ids, len(device_ids))
        else:
            rc = lib.axon_start_nrt_profile(None, 0)
        if rc != 0:
            raise RuntimeError(f"axon_start_nrt_profile rc={rc}")
        try:
            yield
        finally:
            n = lib.axon_stop_nrt_profile(str(output_dir).encode())
            print(f"profile: {n} file(s) written to {output_dir}", file=sys.stderr)

    mod = types.ModuleType("antenv.axon_hooks")
    mod.get_axon_ntff_profile_hook = lambda: _hook
    mod.set_axon_ntff_profile_hook = lambda h: None
    sys.modules["antenv.axon_hooks"] = mod

    import concourse.bass_utils as bu

    bu.upload_artifacts = lambda tmpdir: "local://" + str(tmpdir)


# revision 5
# speedup vs baseline: 1.6280x; 1.6280x over previous
"""Trainium2 Bass kernel for nn_NodeBlock (gnn_message_passing).

reference semantics:
    agg_mesh  = segment_sum(edge_attr, receivers, N)
    agg_world = segment_sum(edge_world_attr, receivers_world, N)
    h = concat([node_attr, agg_mesh, agg_world], -1)   # [N, 3D]
    h = relu(h @ W1 + b1) @ W2 + b2
    out = layernorm(h) * gamma + beta

Strategy (8 cores, nodes sharded by owner, edges partitioned by receiver
owner per the graph-partitioning hint):
  - 256-node supertiles; edges sorted by receiver and packed on host into
    per-supertile 128-row chunks (partition-major layout so DMA moves
    multi-KB contiguous runs per partition).
  - scatter = sum over chunks of edge_chunk^T @ P, accumulated transposed
    [D, node] in PSUM. Edges ship as bf16 hi + bf16 lo (exact to ~1e-5;
    2x faster PE + FWL weight loads vs fp32). P is one-hot built once per
    (supertile, stream) with a single broadcast tensor_tensor is_equal,
    narrowed to a per-chunk node window [base_c, base_c+W_c) shared across
    cores (sorted edges span few nodes per chunk). A zeroing matmul
    initializes the full PSUM tile so window gaps stay zero.
  - MLP in fp32: y^T = sum_j W1_j^T h_j^T directly from the transposed
    aggregates (node_attr arrives pre-transposed); z via lhsT=relu(y^T)
    against W2 augmented with a row-sum column so the LayerNorm mean falls
    out of the matmul for free; LN along the free dim with ACT-side
    square/accumulate and a fused tensor_scalar epilogue.
"""

import os

import numpy as np

LN_EPS = 1e-5
NC_CORES = 8
P = 128
SUP = 256  # nodes per supertile


def _build_program(cfg):
    import concourse.bass as bass
    import concourse.bacc as bacc
    import concourse.tile as tile
    from concourse import mybir

    f32 = mybir.dt.float32
    bf16 = mybir.dt.bfloat16
    TS = cfg["TS"]
    D = cfg["D"]
    NPAD = TS * SUP
    CHm, CHw = cfg["CHm"], cfg["CHw"]          # per-supertile chunk counts
    offm, offw = cfg["offm"], cfg["offw"]      # col offsets (in chunks)
    basem, basew = cfg["basem"], cfg["basew"]  # [s][c] window base
    wm, ww = cfg["wm"], cfg["ww"]              # [s][c] window width
    Wmax = cfg["Wmax"]
    TCm, TCw = offm[-1], offw[-1]              # total chunks per stream
    triv_affine = cfg["triv_affine"]
    b2mean = cfg["b2mean"]

    nc = bacc.Bacc("TRN2")

    emh = nc.dram_tensor("emh", [P, TCm * P], bf16, kind="ExternalInput")
    eml = nc.dram_tensor("eml", [P, TCm * P], bf16, kind="ExternalInput")
    ewh = nc.dram_tensor("ewh", [P, TCw * P], bf16, kind="ExternalInput")
    ewl = nc.dram_tensor("ewl", [P, TCw * P], bf16, kind="ExternalInput")
    rm = nc.dram_tensor("rm", [P, TCm], bf16, kind="ExternalInput")
    rw = nc.dram_tensor("rw", [P, TCw], bf16, kind="ExternalInput")
    ndT = nc.dram_tensor("ndT", [P, NPAD], f32, kind="ExternalInput")
    w1 = nc.dram_tensor("w1", [3 * P, D], f32, kind="ExternalInput")
    w2e = nc.dram_tensor("w2e", [P, D + 1], f32, kind="ExternalInput")
    b1 = nc.dram_tensor("b1", [P, 1], f32, kind="ExternalInput")
    iot = nc.dram_tensor("iot", [P, Wmax], bf16, kind="ExternalInput")
    zer = nc.dram_tensor("zer", [P, SUP], bf16, kind="ExternalInput")
    if not triv_affine:
        gb = nc.dram_tensor("gb", [P, D], f32, kind="ExternalInput")
        bb = nc.dram_tensor("bb", [P, D], f32, kind="ExternalInput")
        b2b = nc.dram_tensor("b2b", [P, D], f32, kind="ExternalInput")
    outd = nc.dram_tensor("out", [NPAD, D], f32, kind="ExternalOutput")

    with tile.TileContext(nc) as tc:
        with (
            tc.tile_pool(name="consts", bufs=1) as consts,
            tc.tile_pool(name="edges", bufs=3) as edges,
            tc.tile_pool(name="ponehot", bufs=3) as ponehot,
            tc.tile_pool(name="aggs", bufs=3) as aggs,
            tc.tile_pool(name="work", bufs=3) as work,
            tc.tile_pool(name="small", bufs=6) as small,
            tc.tile_pool(name="psum", bufs=2, space="PSUM") as psum,
        ):
            w1s = consts.tile([P, 3, D], f32)
            nc.sync.dma_start(out=w1s, in_=w1[:, :].rearrange("(j p) d -> p j d", p=P))
            w2s = consts.tile([P, D + 1], f32)
            nc.sync.dma_start(out=w2s, in_=w2e[:, :])
            b1s = consts.tile([P, 1], f32)
            nc.sync.dma_start(out=b1s, in_=b1[:, :])
            iots = consts.tile([P, Wmax], bf16)
            nc.sync.dma_start(out=iots, in_=iot[:, :])
            zers = consts.tile([P, SUP], bf16)
            nc.sync.dma_start(out=zers, in_=zer[:, :])
            epss = consts.tile([P, 1], f32)
            nc.vector.memset(epss, LN_EPS)
            ndTs = consts.tile([P, NPAD], f32)
            nc.sync.dma_start(out=ndTs, in_=ndT[:, :])
            rms = consts.tile([P, TCm], bf16)
            nc.sync.dma_start(out=rms, in_=rm[:, :])
            rws = consts.tile([P, TCw], bf16)
            nc.sync.dma_start(out=rws, in_=rw[:, :])
            if not triv_affine:
                gbs = consts.tile([P, D], f32)
                nc.sync.dma_start(out=gbs, in_=gb[:, :])
                bbs = consts.tile([P, D], f32)
                nc.sync.dma_start(out=bbs, in_=bb[:, :])
                b2s = consts.tile([P, D], f32)
                nc.sync.dma_start(out=b2s, in_=b2b[:, :])

            def onehot(r_all, off_s, ch_s, tag):
                """P[e, c, n] = (iota[n] == r[e, c]) in one broadcast TT."""
                pm = ponehot.tile([P, ch_s, Wmax], bf16, tag=tag)
                r_sl = r_all[:, off_s : off_s + ch_s]
                r_b = bass.AP(
                    tensor=r_sl.tensor,
                    offset=r_sl.offset,
                    ap=[r_sl.ap[0], r_sl.ap[1], [0, Wmax]],
                )
                i_sl = iots[:, :]
                i_b = bass.AP(
                    tensor=i_sl.tensor,
                    offset=i_sl.offset,
                    ap=[i_sl.ap[0], [0, ch_s], i_sl.ap[1]],
                )
                nc.vector.tensor_tensor(
                    out=pm, in0=r_b, in1=i_b, op=mybir.AluOpType.is_equal
                )
                return pm

            def scatter(eh_t, el_t, pm, ps, ch_s, base_s, w_s):
                # init full tile to zero (sets has_written) then accumulate
                nc.tensor.matmul(
                    out=ps,
                    lhsT=zers[:, :P],
                    rhs=zers[:, :],
                    start=True,
                    stop=False,
                    skip_group_check=True,
                )
                for c in range(ch_s):
                    for op_t in (eh_t, el_t):
                        nc.tensor.matmul(
                            out=ps[:, base_s[c] : base_s[c] + w_s[c]],
                            lhsT=op_t[:, c * P : (c + 1) * P],
                            rhs=pm[:, c, 0 : w_s[c]],
                            start=False,
                            stop=(c == ch_s - 1) and (op_t is el_t),
                            skip_group_check=True,
                        )

            for s in range(TS):
                chm, chw = CHm[s], CHw[s]
                emh_t = edges.tile([P, chm * P], bf16, tag="emh")
                nc.sync.dma_start(
                    out=emh_t, in_=emh[:, offm[s] * P : (offm[s] + chm) * P]
                )
                eml_t = edges.tile([P, chm * P], bf16, tag="eml")
                nc.sync.dma_start(
                    out=eml_t, in_=eml[:, offm[s] * P : (offm[s] + chm) * P]
                )
                ewh_t = edges.tile([P, chw * P], bf16, tag="ewh")
                nc.sync.dma_start(
                    out=ewh_t, in_=ewh[:, offw[s] * P : (offw[s] + chw) * P]
                )
                ewl_t = edges.tile([P, chw * P], bf16, tag="ewl")
                nc.sync.dma_start(
                    out=ewl_t, in_=ewl[:, offw[s] * P : (offw[s] + chw) * P]
                )

                pm = onehot(rms, offm[s], chm, "pm")
                pw = onehot(rws, offw[s], chw, "pw")

                aggm_ps = psum.tile([P, SUP], f32, tag="aggm")
                scatter(emh_t, eml_t, pm, aggm_ps, chm, basem[s], wm[s])
                aggw_ps = psum.tile([P, SUP], f32, tag="aggw")
                scatter(ewh_t, ewl_t, pw, aggw_ps, chw, basew[s], ww[s])

                aggms = aggs.tile([P, SUP], f32, tag="aggms")
                nc.any.tensor_copy(out=aggms, in_=aggm_ps)
                aggws = aggs.tile([P, SUP], f32, tag="aggws")
                nc.any.tensor_copy(out=aggws, in_=aggw_ps)

                # MLP: y^T[dout, n] = sum_k W1[k, dout] h^T[k, n]
                y_ps = psum.tile([P, SUP], f32, tag="y")
                nc.tensor.matmul(
                    out=y_ps,
                    lhsT=w1s[:, 0, :],
                    rhs=ndTs[:, s * SUP : (s + 1) * SUP],
                    start=True,
                    stop=False,
                )
                nc.tensor.matmul(
                    out=y_ps, lhsT=w1s[:, 1, :], rhs=aggms, start=False, stop=False
                )
                nc.tensor.matmul(
                    out=y_ps, lhsT=w1s[:, 2, :], rhs=aggws, start=False, stop=True
                )
                yr = work.tile([P, SUP], f32, tag="yr")
                nc.scalar.activation(
                    out=yr,
                    in_=y_ps,
                    func=mybir.ActivationFunctionType.Relu,
                    bias=b1s[:, :],
                    scale=1.0,
                )
                # z_ext[n, :D] = z, z_ext[n, D] = sum_dz z  (for the mean)
                z_ps = psum.tile([P, 2, D + 1], f32, tag="z")
                for h in range(2):
                    nc.tensor.matmul(
                        out=z_ps[:, h, :],
                        lhsT=yr[:, h * P : (h + 1) * P],
                        rhs=w2s,
                        start=True,
                        stop=True,
                    )
                for h in range(2):
                    zh = z_ps[:, h, 0:D]
                    negmu = small.tile([P, 1], f32, tag="negmu")
                    nc.scalar.activation(
                        out=negmu,
                        in_=z_ps[:, h, D : D + 1],
                        func=mybir.ActivationFunctionType.Copy,
                        bias=-b2mean,
                        scale=-1.0 / D,
                    )
                    sq = work.tile([P, D], f32, tag="sq")
                    sqsum = small.tile([P, 1], f32, tag="sqsum")
                    if triv_affine:
                        zc = zh  # centered input is z + negmu via ACT bias
                        nc.scalar.activation(
                            out=sq,
                            in_=zh,
                            func=mybir.ActivationFunctionType.Square,
                            bias=negmu[:, :],
                            scale=1.0,
                            accum_out=sqsum,
                        )
                    else:
                        # t0 = z + b2 - mu (materialized; feeds var and out)
                        t0 = work.tile([P, D], f32, tag="t0")
                        nc.vector.scalar_tensor_tensor(
                            out=t0,
                            in0=zh,
                            scalar=negmu,
                            in1=b2s,
                            op0=mybir.AluOpType.add,
                            op1=mybir.AluOpType.add,
                        )
                        nc.scalar.activation(
                            out=sq,
                            in_=t0,
                            func=mybir.ActivationFunctionType.Square,
                            accum_out=sqsum,
                        )
                    std = small.tile([P, 1], f32, tag="std")
                    nc.scalar.activation(
                        out=std,
                        in_=sqsum,
                        func=mybir.ActivationFunctionType.Sqrt,
                        bias=epss[:, :],
                        scale=1.0 / D,
                    )
                    rstd = small.tile([P, 1], f32, tag="rstd")
                    nc.vector.reciprocal(out=rstd, in_=std)
                    outt = work.tile([P, D], f32, tag="outt")
                    if triv_affine:
                        nc.vector.tensor_scalar(
                            out=outt,
                            in0=zh,
                            scalar1=negmu,
                            scalar2=rstd,
                            op0=mybir.AluOpType.add,
                            op1=mybir.AluOpType.mult,
                        )
                    else:
                        # (z + b2 - mu) * rstd * gamma + beta
                        t1 = work.tile([P, D], f32, tag="t1")
                        nc.vector.scalar_tensor_tensor(
                            out=t1,
                            in0=t0,
                            scalar=rstd,
                            in1=gbs,
                            op0=mybir.AluOpType.mult,
                            op1=mybir.AluOpType.mult,
                        )
                        nc.vector.tensor_tensor(
                            out=outt, in0=t1, in1=bbs, op=mybir.AluOpType.add
                        )
                    row = s * SUP + h * P
                    nc.sync.dma_start(out=outd[row : row + P, :], in_=outt)

    nc.finalize()
    return nc


def _pack_edges(edge_attr, recv, NPC, TS, n_cores, bf16):
    """Sort by receiver, partition by owner core, pack into per-supertile
    128-row chunks in partition-major bf16 hi/lo layout with shared
    per-chunk node windows."""
    D = edge_attr.shape[1]
    order = np.argsort(recv, kind="stable")
    r_sorted = recv[order]
    c_ids = r_sorted // NPC
    loc = r_sorted - c_ids * NPC
    s_ids = loc // SUP
    rl = loc - s_ids * SUP
    g_ids = c_ids * TS + s_ids
    cnt = np.bincount(g_ids, minlength=n_cores * TS)
    starts = np.cumsum(cnt) - cnt
    pos = np.arange(len(r_sorted)) - starts[g_ids]
    ch_ids = pos // P

    # per-supertile chunk count = max over cores
    CH = np.maximum(
        1, np.ceil(cnt.reshape(n_cores, TS) / P).astype(np.int64).max(axis=0)
    )  # [TS]
    off = np.concatenate([[0], np.cumsum(CH)]).astype(np.int64)  # [TS+1]
    TC = int(off[-1])

    # shared window base/width per (supertile, chunk)
    CHmax = int(CH.max())
    minrl = np.full((n_cores * TS, CHmax), 10**9, np.int64)
    maxrl = np.full((n_cores * TS, CHmax), -1, np.int64)
    idx = g_ids * CHmax + ch_ids
    np.minimum.at(minrl.reshape(-1), idx, rl)
    np.maximum.at(maxrl.reshape(-1), idx, rl)
    minrl = minrl.reshape(n_cores, TS, CHmax).min(axis=0)  # [TS, CHmax]
    maxrl = maxrl.reshape(n_cores, TS, CHmax).max(axis=0)
    base = np.clip(minrl, 0, SUP)
    width = np.clip(maxrl - base + 1, 1, SUP)
    base_l = [[int(base[s, c]) for c in range(CH[s])] for s in range(TS)]
    w_l = [[int(width[s, c]) for c in range(CH[s])] for s in range(TS)]
    Wmax = max(1, int(max(max(w) for w in w_l)))
    assert Wmax <= 200

    # destination column in the [P, TC*P] partition-major buffer:
    # supertile s, chunk c, edge e (partition), dim d -> [e, (off[s]+c)*P + d]
    part = pos - ch_ids * P  # partition = edge index within chunk
    colchunk = off[s_ids] + ch_ids  # global chunk column
    ea = edge_attr[order].astype(np.float32)
    hi = ea.astype(bf16)
    lo = (ea - hi.astype(np.float32)).astype(bf16)

    emh = np.zeros((n_cores, P, TC * P), bf16)
    eml = np.zeros((n_cores, P, TC * P), bf16)
    rlp = np.full((n_cores, P, TC), 250.0, bf16)  # sentinel: never matches
    cols = (colchunk * P)[:, None] + np.arange(D)[None, :]
    emh[c_ids[:, None], part[:, None], cols] = hi
    eml[c_ids[:, None], part[:, None], cols] = lo
    rl_rel = rl - base[s_ids, ch_ids]
    rlp[c_ids, part, colchunk] = rl_rel.astype(bf16)

    return emh, eml, rlp, [int(x) for x in CH], [int(x) for x in off], base_l, w_l, Wmax


def kernel(**inputs):
    import ml_dtypes
    from concourse.bass_utils import run_bass_kernel_spmd

    bf16 = np.dtype(ml_dtypes.bfloat16)

    node_attr = np.asarray(inputs["node_attr"], np.float32)
    edge_attr = np.asarray(inputs["edge_attr"], np.float32)
    edge_world_attr = np.asarray(inputs["edge_world_attr"], np.float32)
    recv = np.asarray(inputs["receivers"]).astype(np.int64)
    recv_w = np.asarray(inputs["receivers_world"]).astype(np.int64)
    W1 = np.asarray(inputs["W1"], np.float32)
    b1 = np.asarray(inputs["b1"], np.float32)
    W2 = np.asarray(inputs["W2"], np.float32)
    b2 = np.asarray(inputs["b2"], np.float32)
    gamma = np.asarray(inputs["gamma"], np.float32)
    beta = np.asarray(inputs["beta"], np.float32)

    N, D = node_attr.shape
    assert D == P and N % NC_CORES == 0
    NPC = N // NC_CORES
    TS = (NPC + SUP - 1) // SUP
    NPAD = TS * SUP

    emh, eml, rmp, CHm, offm, basem, wm, Wm = _pack_edges(
        edge_attr, recv, NPC, TS, NC_CORES, bf16
    )
    ewh, ewl, rwp, CHw, offw, basew, ww, Ww = _pack_edges(
        edge_world_attr, recv_w, NPC, TS, NC_CORES, bf16
    )
    Wmax = max(Wm, Ww)

    ndT = np.zeros((NC_CORES, P, NPAD), np.float32)
    for c in range(NC_CORES):
        ndT[c, :, :NPC] = node_attr[c * NPC : (c + 1) * NPC].T

    triv_affine = (
        not b2.any() and not beta.any() and bool(np.all(gamma == 1.0))
    )
    cfg = {
        "TS": TS,
        "D": D,
        "CHm": CHm,
        "CHw": CHw,
        "offm": offm,
        "offw": offw,
        "basem": basem,
        "basew": basew,
        "wm": wm,
        "ww": ww,
        "Wmax": Wmax,
        "triv_affine": triv_affine,
        "b2mean": float(b2.mean()),
    }
    nc = _build_program(cfg)

    iota = np.tile(np.arange(Wmax, dtype=np.float32), (P, 1)).astype(bf16)
    zeros_b = np.zeros((P, SUP), bf16)
    w2e = np.concatenate([W2, W2.sum(axis=1, keepdims=True)], axis=1).astype(
        np.float32
    )
    b1c = np.ascontiguousarray(b1.reshape(P, 1))

    in_maps = []
    for c in range(NC_CORES):
        m = {
            "emh": emh[c],
            "eml": eml[c],
            "ewh": ewh[c],
            "ewl": ewl[c],
            "rm": rmp[c],
            "rw": rwp[c],
            "ndT": ndT[c],
            "w1": W1,
            "w2e": w2e,
            "b1": b1c,
            "iot": iota,
            "zer": zeros_b,
        }
        if not triv_affine:
            m["gb"] = np.tile(gamma, (P, 1)).astype(np.float32)
            m["bb"] = np.tile(beta, (P, 1)).astype(np.float32)
            m["b2b"] = np.tile(b2, (P, 1)).astype(np.float32)
        in_maps.append(m)

    prof_dir = os.environ.get("KERNEL_PROFILE_DIR")
    trace = False
    if prof_dir:
        try:
            _install_profile_hook()
            trace = True
        except Exception as e:  # profiling is best-effort
            print(f"profile hook unavailable: {e}")

    res = run_bass_kernel_spmd(
        nc,
        in_maps,
        core_ids=list(range(NC_CORES)),
        trace=trace,
        tmpdir=prof_dir if trace else None,
    )
    if trace:
        print(f"HW exec time: {res.exec_time_ns} ns")

    out = np.concatenate(
        [res.results[c]["out"][:NPC] for c in range(NC_CORES)], axis=0
    )
    return out


def _install_profile_hook():
    """Register the axon NTFF profile hook (the boot path skips it when
    antenv.axon_hooks is absent) and neuter the artifact upload."""
    import contextlib
    import ctypes
    import sys
    import types

    lib = ctypes.CDLL("/opt/axon/libaxon_pjrt.so")
    lib.axon_start_nrt_profile.argtypes = [
        ctypes.POINTER(ctypes.c_int64),
        ctypes.c_size_t,
    ]
    lib.axon_start_nrt_profile.restype = ctypes.c_int64
    lib.axon_stop_nrt_profile.argtypes = [ctypes.c_char_p]
    lib.axon_stop_nrt_profile.restype = ctypes.c_int64

    @contextlib.contextmanager
    def _hook(output_dir, device_ids):
        import jax

        jax.devices()
        if device_ids:
            ids = (ctypes.c_int64 * len(device_ids))(*device_ids)
            rc = lib.axon_start_nrt_profile(ids, len(device_ids))
        else:
            rc = lib.axon_start_nrt_profile(None, 0)
        if rc != 0:
            raise RuntimeError(f"axon_start_nrt_profile rc={rc}")
        try:
            yield
        finally:
            n = lib.axon_stop_nrt_profile(str(output_dir).encode())
            print(f"profile: {n} file(s) written to {output_dir}", file=sys.stderr)

    mod = types.ModuleType("antenv.axon_hooks")
    mod.get_axon_ntff_profile_hook = lambda: _hook
    mod.set_axon_ntff_profile_hook = lambda h: None
    sys.modules["antenv.axon_hooks"] = mod

    import concourse.bass_utils as bu

    bu.upload_artifacts = lambda tmpdir: "local://" + str(tmpdir)


# revision 9
# speedup vs baseline: 1.6435x; 1.0096x over previous
"""Trainium2 Bass kernel for nn_NodeBlock (gnn_message_passing).

reference semantics:
    agg_mesh  = segment_sum(edge_attr, receivers, N)
    agg_world = segment_sum(edge_world_attr, receivers_world, N)
    h = concat([node_attr, agg_mesh, agg_world], -1)   # [N, 3D]
    h = relu(h @ W1 + b1) @ W2 + b2
    out = layernorm(h) * gamma + beta

Strategy (8 cores, nodes sharded by owner, edges partitioned by receiver
owner per the graph-partitioning hint):
  - 256-node supertiles; edges sorted by receiver and packed on host into
    per-supertile 128-row chunks (partition-major layout so DMA moves
    multi-KB contiguous runs per partition).
  - scatter = sum over chunks of edge_chunk^T @ P, accumulated transposed
    [D, node] in PSUM. Edges ship as bf16 hi + bf16 lo (exact to ~1e-5;
    2x faster PE + FWL weight loads vs fp32). P is one-hot built once per
    (supertile, stream) with a single broadcast tensor_tensor is_equal,
    narrowed to a per-chunk node window [base_c, base_c+W_c) shared across
    cores (sorted edges span few nodes per chunk). A zeroing matmul
    initializes the full PSUM tile so window gaps stay zero.
  - MLP in fp32: y^T = sum_j W1_j^T h_j^T directly from the transposed
    aggregates (node_attr arrives pre-transposed); z via lhsT=relu(y^T)
    against W2 augmented with a row-sum column so the LayerNorm mean falls
    out of the matmul for free; LN along the free dim with ACT-side
    square/accumulate and a fused tensor_scalar epilogue.
"""

import os

import numpy as np

LN_EPS = 1e-5
NC_CORES = 8
P = 128
SUP = 256  # nodes per supertile


def _build_program(cfg):
    import concourse.bass as bass
    import concourse.bacc as bacc
    import concourse.tile as tile
    from concourse import mybir

    f32 = mybir.dt.float32
    bf16 = mybir.dt.bfloat16
    TS = cfg["TS"]
    D = cfg["D"]
    NPAD = TS * SUP
    CHm, CHw = cfg["CHm"], cfg["CHw"]          # per-supertile chunk counts
    offm, offw = cfg["offm"], cfg["offw"]      # col offsets (in chunks)
    basem, basew = cfg["basem"], cfg["basew"]  # [s][c] window base
    wm, ww = cfg["wm"], cfg["ww"]              # [s][c] window width
    Wmax = cfg["Wmax"]
    TCm, TCw = offm[-1], offw[-1]              # total chunks per stream
    triv_affine = cfg["triv_affine"]
    b2mean = cfg["b2mean"]

    f32r = mybir.dt.float32r if cfg["use_f32r"] else mybir.dt.float32
    nc = bacc.Bacc("TRN2")

    emh = nc.dram_tensor("emh", [P, TCm * P], bf16, kind="ExternalInput")
    eml = nc.dram_tensor("eml", [P, TCm * P], bf16, kind="ExternalInput")
    ewh = nc.dram_tensor("ewh", [P, TCw * P], bf16, kind="ExternalInput")
    ewl = nc.dram_tensor("ewl", [P, TCw * P], bf16, kind="ExternalInput")
    rm = nc.dram_tensor("rm", [P, TCm], bf16, kind="ExternalInput")
    rw = nc.dram_tensor("rw", [P, TCw], bf16, kind="ExternalInput")
    ndT = nc.dram_tensor("ndT", [P, NPAD], f32r, kind="ExternalInput")
    w1 = nc.dram_tensor("w1", [3 * P, D], f32r, kind="ExternalInput")
    w2e = nc.dram_tensor("w2e", [P, D + 2], f32r, kind="ExternalInput")
    b1 = nc.dram_tensor("b1", [P, 1], f32, kind="ExternalInput")
    iot = nc.dram_tensor("iot", [P, Wmax], bf16, kind="ExternalInput")
    zer = nc.dram_tensor("zer", [P, 2 * SUP], bf16, kind="ExternalInput")
    if not triv_affine:
        gb = nc.dram_tensor("gb", [P, D], f32, kind="ExternalInput")
        bb = nc.dram_tensor("bb", [P, D], f32, kind="ExternalInput")
        b2b = nc.dram_tensor("b2b", [P, D], f32, kind="ExternalInput")
    outd = nc.dram_tensor("out", [NPAD, D], f32, kind="ExternalOutput")

    with tile.TileContext(nc) as tc:
        with (
            tc.tile_pool(name="consts", bufs=1) as consts,
            tc.tile_pool(name="edges", bufs=4) as edges,
            tc.tile_pool(name="ponehot", bufs=3) as ponehot,
            tc.tile_pool(name="aggs", bufs=3) as aggs,
            tc.tile_pool(name="work", bufs=3) as work,
            tc.tile_pool(name="small", bufs=6) as small,
            tc.tile_pool(name="psag", bufs=3, space="PSUM") as psag,
            tc.tile_pool(name="psy", bufs=2, space="PSUM") as psy,
            tc.tile_pool(name="psz", bufs=2, space="PSUM") as psz,
        ):
            w1s = consts.tile([P, 3, D], f32r)
            nc.sync.dma_start(out=w1s, in_=w1[:, :].rearrange("(j p) d -> p j d", p=P))
            w2s = consts.tile([P, D + 2], f32r)
            nc.sync.dma_start(out=w2s, in_=w2e[:, :])
            b1s = consts.tile([P, 1], f32)
            nc.sync.dma_start(out=b1s, in_=b1[:, :])
            iots = consts.tile([P, Wmax], bf16)
            nc.sync.dma_start(out=iots, in_=iot[:, :])
            zers = consts.tile([P, 2 * SUP], bf16)
            nc.sync.dma_start(out=zers, in_=zer[:, :])
            epss = consts.tile([P, 1], f32)
            nc.vector.memset(epss, LN_EPS)
            ndTs = consts.tile([P, NPAD], f32r)
            nc.sync.dma_start(out=ndTs, in_=ndT[:, :])
            rms = consts.tile([P, TCm], bf16)
            nc.sync.dma_start(out=rms, in_=rm[:, :])
            rws = consts.tile([P, TCw], bf16)
            nc.sync.dma_start(out=rws, in_=rw[:, :])
            if not triv_affine:
                gbs = consts.tile([P, D], f32)
                nc.sync.dma_start(out=gbs, in_=gb[:, :])
                bbs = consts.tile([P, D], f32)
                nc.sync.dma_start(out=bbs, in_=bb[:, :])
                b2s = consts.tile([P, D], f32)
                nc.sync.dma_start(out=b2s, in_=b2b[:, :])

            def onehot(r_all, off_s, ch_s, tag):
                """P[e, c, n] = (iota[n] == r[e, c]) in one broadcast TT."""
                pm = ponehot.tile([P, ch_s, Wmax], bf16, tag=tag)
                r_sl = r_all[:, off_s : off_s + ch_s]
                r_b = bass.AP(
                    tensor=r_sl.tensor,
                    offset=r_sl.offset,
                    ap=[r_sl.ap[0], r_sl.ap[1], [0, Wmax]],
                )
                i_sl = iots[:, :]
                i_b = bass.AP(
                    tensor=i_sl.tensor,
                    offset=i_sl.offset,
                    ap=[i_sl.ap[0], [0, ch_s], i_sl.ap[1]],
                )
                nc.vector.tensor_tensor(
                    out=pm, in0=r_b, in1=i_b, op=mybir.AluOpType.is_equal
                )
                return pm

            def scatter(eh_t, el_t, pm, ps, col0, ch_s, base_s, w_s, last):
                for c in range(ch_s):
                    for op_t in (eh_t, el_t):
                        b = col0 + base_s[c]
                        nc.tensor.matmul(
                            out=ps[:, b : b + w_s[c]],
                            lhsT=op_t[:, c * P : (c + 1) * P],
                            rhs=pm[:, c, 0 : w_s[c]],
                            start=False,
                            stop=last and (c == ch_s - 1) and (op_t is el_t),
                            skip_group_check=True,
                        )

            for s in range(TS):
                chm, chw = CHm[s], CHw[s]
                emh_t = edges.tile([P, chm * P], bf16, tag="emh")
                nc.sync.dma_start(
                    out=emh_t, in_=emh[:, offm[s] * P : (offm[s] + chm) * P]
                )
                eml_t = edges.tile([P, chm * P], bf16, tag="eml")
                nc.sync.dma_start(
                    out=eml_t, in_=eml[:, offm[s] * P : (offm[s] + chm) * P]
                )
                ewh_t = edges.tile([P, chw * P], bf16, tag="ewh")
                nc.sync.dma_start(
                    out=ewh_t, in_=ewh[:, offw[s] * P : (offw[s] + chw) * P]
                )
                ewl_t = edges.tile([P, chw * P], bf16, tag="ewl")
                nc.sync.dma_start(
                    out=ewl_t, in_=ewl[:, offw[s] * P : (offw[s] + chw) * P]
                )

                pm = onehot(rms, offm[s], chm, "pm")
                pw = onehot(rws, offw[s], chw, "pw")

                agg_ps = psag.tile([P, 2 * SUP], f32, tag="agg")
                nc.tensor.matmul(
                    out=agg_ps,
                    lhsT=zers[:, :P],
                    rhs=zers[:, :],
                    start=True,
                    stop=False,
                    skip_group_check=True,
                )
                scatter(emh_t, eml_t, pm, agg_ps, 0, chm, basem[s], wm[s], False)
                scatter(ewh_t, ewl_t, pw, agg_ps, SUP, chw, basew[s], ww[s], True)

                aggsb = aggs.tile([P, 2 * SUP], f32r, tag="aggsb")
                nc.any.tensor_copy(out=aggsb, in_=agg_ps)

                # MLP: y^T[dout, n] = sum_k W1[k, dout] h^T[k, n]
                y_ps = psy.tile([P, SUP], f32, tag="y")
                nc.tensor.matmul(
                    out=y_ps,
                    lhsT=w1s[:, 0, :],
                    rhs=ndTs[:, s * SUP : (s + 1) * SUP],
                    start=True,
                    stop=False,
                )
                nc.tensor.matmul(
                    out=y_ps,
                    lhsT=w1s[:, 1, :],
                    rhs=aggsb[:, 0:SUP],
                    start=False,
                    stop=False,
                )
                nc.tensor.matmul(
                    out=y_ps, lhsT=w1s[:, 2, :], rhs=aggsb[:, SUP : 2 * SUP], start=False, stop=True
                )
                yr = work.tile([P, SUP], f32r, tag="yr")
                nc.scalar.activation(
                    out=yr,
                    in_=y_ps,
                    func=mybir.ActivationFunctionType.Relu,
                    bias=b1s[:, :],
                    scale=1.0,
                )
                # z_ext[n, :D] = z, z_ext[n, D] = sum_dz z  (for the mean)
                z_ps = psz.tile([P, 2, D + 2], f32, tag="z")
                for h in range(2):
                    nc.tensor.matmul(
                        out=z_ps[:, h, :],
                        lhsT=yr[:, h * P : (h + 1) * P],
                        rhs=w2s,
                        start=True,
                        stop=True,
                    )
                for h in range(2):
                    zh = z_ps[:, h, 0:D]
                    negmu = small.tile([P, 1], f32, tag="negmu")
                    nc.scalar.activation(
                        out=negmu,
                        in_=z_ps[:, h, D : D + 1],
                        func=mybir.ActivationFunctionType.Copy,
                        bias=-b2mean,
                        scale=-1.0 / D,
                    )
                    sq = work.tile([P, D], f32, tag="sq")
                    sqsum = small.tile([P, 1], f32, tag="sqsum")
                    if triv_affine:
                        zc = zh  # centered input is z + negmu via ACT bias
                        nc.scalar.activation(
                            out=sq,
                            in_=zh,
                            func=mybir.ActivationFunctionType.Square,
                            bias=negmu[:, :],
                            scale=1.0,
                            accum_out=sqsum,
                        )
                    else:
                        # t0 = z + b2 - mu (materialized; feeds var and out)
                        t0 = work.tile([P, D], f32, tag="t0")
                        nc.vector.scalar_tensor_tensor(
                            out=t0,
                            in0=zh,
                            scalar=negmu,
                            in1=b2s,
                            op0=mybir.AluOpType.add,
                            op1=mybir.AluOpType.add,
                        )
                        nc.scalar.activation(
                            out=sq,
                            in_=t0,
                            func=mybir.ActivationFunctionType.Square,
                            accum_out=sqsum,
                        )
                    std = small.tile([P, 1], f32, tag="std")
                    nc.scalar.activation(
                        out=std,
                        in_=sqsum,
                        func=mybir.ActivationFunctionType.Sqrt,
                        bias=epss[:, :],
                        scale=1.0 / D,
                    )
                    rstd = small.tile([P, 1], f32, tag="rstd")
                    nc.vector.reciprocal(out=rstd, in_=std)
                    outt = work.tile([P, D], f32, tag="outt")
                    if triv_affine:
                        nc.vector.tensor_scalar(
                            out=outt,
                            in0=zh,
                            scalar1=negmu,
                            scalar2=rstd,
                            op0=mybir.AluOpType.add,
                            op1=mybir.AluOpType.mult,
                        )
                    else:
                        # (z + b2 - mu) * rstd * gamma + beta
                        t1 = work.tile([P, D], f32, tag="t1")
                        nc.vector.scalar_tensor_tensor(
                            out=t1,
                            in0=t0,
                            scalar=rstd,
                            in1=gbs,
                            op0=mybir.AluOpType.mult,
                            op1=mybir.AluOpType.mult,
                        )
                        nc.vector.tensor_tensor(
                            out=outt, in0=t1, in1=bbs, op=mybir.AluOpType.add
                        )
                    row = s * SUP + h * P
                    nc.sync.dma_start(out=outd[row : row + P, :], in_=outt)

    nc.finalize()
    return nc


def _pack_edges(edge_attr, recv, NPC, TS, n_cores, bf16):
    """Sort by receiver, partition by owner core, pack into per-supertile
    128-row chunks in partition-major bf16 hi/lo layout with shared
    per-chunk node windows."""
    D = edge_attr.shape[1]
    order = np.argsort(recv, kind="stable")
    r_sorted = recv[order]
    c_ids = r_sorted // NPC
    loc = r_sorted - c_ids * NPC
    s_ids = loc // SUP
    rl = loc - s_ids * SUP
    g_ids = c_ids * TS + s_ids
    cnt = np.bincount(g_ids, minlength=n_cores * TS)
    starts = np.cumsum(cnt) - cnt
    pos = np.arange(len(r_sorted)) - starts[g_ids]
    ch_ids = pos // P

    # per-supertile chunk count = max over cores
    CH = np.maximum(
        1, np.ceil(cnt.reshape(n_cores, TS) / P).astype(np.int64).max(axis=0)
    )  # [TS]
    off = np.concatenate([[0], np.cumsum(CH)]).astype(np.int64)  # [TS+1]
    TC = int(off[-1])

    # shared window base/width per (supertile, chunk)
    CHmax = int(CH.max())
    minrl = np.full((n_cores * TS, CHmax), 10**9, np.int64)
    maxrl = np.full((n_cores * TS, CHmax), -1, np.int64)
    idx = g_ids * CHmax + ch_ids
    np.minimum.at(minrl.reshape(-1), idx, rl)
    np.maximum.at(maxrl.reshape(-1), idx, rl)
    minrl = minrl.reshape(n_cores, TS, CHmax).min(axis=0)  # [TS, CHmax]
    maxrl = maxrl.reshape(n_cores, TS, CHmax).max(axis=0)
    base = np.clip(minrl, 0, SUP)
    width = np.clip(maxrl - base + 1, 1, SUP)
    base_l = [[int(base[s, c]) for c in range(CH[s])] for s in range(TS)]
    w_l = [[int(width[s, c]) for c in range(CH[s])] for s in range(TS)]
    Wmax = max(1, int(max(max(w) for w in w_l)))
    assert Wmax <= 200

    # destination column in the [P, TC*P] partition-major buffer:
    # supertile s, chunk c, edge e (partition), dim d -> [e, (off[s]+c)*P + d]
    part = pos - ch_ids * P  # partition = edge index within chunk
    colchunk = off[s_ids] + ch_ids  # global chunk column
    ea = edge_attr[order].astype(np.float32)
    hi = ea.astype(bf16)
    lo = (ea - hi.astype(np.float32)).astype(bf16)

    emh = np.zeros((n_cores, P, TC * P), bf16)
    eml = np.zeros((n_cores, P, TC * P), bf16)
    rlp = np.full((n_cores, P, TC), 250.0, bf16)  # sentinel: never matches
    cols = (colchunk * P)[:, None] + np.arange(D)[None, :]
    emh[c_ids[:, None], part[:, None], cols] = hi
    eml[c_ids[:, None], part[:, None], cols] = lo
    rl_rel = rl - base[s_ids, ch_ids]
    rlp[c_ids, part, colchunk] = rl_rel.astype(bf16)

    return emh, eml, rlp, [int(x) for x in CH], [int(x) for x in off], base_l, w_l, Wmax


def kernel(**inputs):
    import ml_dtypes
    from concourse.bass_utils import run_bass_kernel_spmd

    bf16 = np.dtype(ml_dtypes.bfloat16)

    node_attr = np.asarray(inputs["node_attr"], np.float32)
    edge_attr = np.asarray(inputs["edge_attr"], np.float32)
    edge_world_attr = np.asarray(inputs["edge_world_attr"], np.float32)
    recv = np.asarray(inputs["receivers"]).astype(np.int64)
    recv_w = np.asarray(inputs["receivers_world"]).astype(np.int64)
    W1 = np.asarray(inputs["W1"], np.float32)
    b1 = np.asarray(inputs["b1"], np.float32)
    W2 = np.asarray(inputs["W2"], np.float32)
    b2 = np.asarray(inputs["b2"], np.float32)
    gamma = np.asarray(inputs["gamma"], np.float32)
    beta = np.asarray(inputs["beta"], np.float32)

    N, D = node_attr.shape
    assert D == P and N % NC_CORES == 0
    NPC = N // NC_CORES
    TS = (NPC + SUP - 1) // SUP
    NPAD = TS * SUP

    emh, eml, rmp, CHm, offm, basem, wm, Wm = _pack_edges(
        edge_attr, recv, NPC, TS, NC_CORES, bf16
    )
    ewh, ewl, rwp, CHw, offw, basew, ww, Ww = _pack_edges(
        edge_world_attr, recv_w, NPC, TS, NC_CORES, bf16
    )
    Wmax = max(Wm, Ww)

    ndT = np.zeros((NC_CORES, P, NPAD), np.float32)
    for c in range(NC_CORES):
        ndT[c, :, :NPC] = node_attr[c * NPC : (c + 1) * NPC].T

    triv_affine = (
        not b2.any() and not beta.any() and bool(np.all(gamma == 1.0))
    )
    cfg = {
        "TS": TS,
        "D": D,
        "CHm": CHm,
        "CHw": CHw,
        "offm": offm,
        "offw": offw,
        "basem": basem,
        "basew": basew,
        "wm": wm,
        "ww": ww,
        "Wmax": Wmax,
        "triv_affine": triv_affine,
        "use_f32r": os.environ.get("KERNEL_F32R", "") != "",
        "b2mean": float(b2.mean()),
    }
    nc = _build_program(cfg)

    iota = np.tile(np.arange(Wmax, dtype=np.float32), (P, 1)).astype(bf16)
    zeros_b = np.zeros((P, 2 * SUP), bf16)
    w2e = np.concatenate(
        [W2, W2.sum(axis=1, keepdims=True), np.zeros((P, 1), np.float32)], axis=1
    ).astype(np.float32)
    b1c = np.ascontiguousarray(b1.reshape(P, 1))

    in_maps = []
    for c in range(NC_CORES):
        m = {
            "emh": emh[c],
            "eml": eml[c],
            "ewh": ewh[c],
            "ewl": ewl[c],
            "rm": rmp[c],
            "rw": rwp[c],
            "ndT": ndT[c],
            "w1": W1,
            "w2e": w2e,
            "b1": b1c,
            "iot": iota,
            "zer": zeros_b,
        }
        if not triv_affine:
            m["gb"] = np.tile(gamma, (P, 1)).astype(np.float32)
            m["bb"] = np.tile(beta, (P, 1)).astype(np.float32)
            m["b2b"] = np.tile(b2, (P, 1)).astype(np.float32)
        in_maps.append(m)

    prof_dir = os.environ.get("KERNEL_PROFILE_DIR")
    trace = False
    if prof_dir:
        try:
            _install_profile_hook()
            trace = True
        except Exception as e:  # profiling is best-effort
            print(f"profile hook unavailable: {e}")

    res = run_bass_kernel_spmd(
        nc,
        in_maps,
        core_ids=list(range(NC_CORES)),
        trace=trace,
        tmpdir=prof_dir if trace else None,
    )
    if trace:
        print(f"HW exec time: {res.exec_time_ns} ns")

    out = np.concatenate(
        [res.results[c]["out"][:NPC] for c in range(NC_CORES)], axis=0
    )
    return out


def _install_profile_hook():
    """Register the axon NTFF profile hook (the boot path skips it when
    antenv.axon_hooks is absent) and neuter the artifact upload."""
    import contextlib
    import ctypes
    import sys
    import types

    lib = ctypes.CDLL("/opt/axon/libaxon_pjrt.so")
    lib.axon_start_nrt_profile.argtypes = [
        ctypes.POINTER(ctypes.c_int64),
        ctypes.c_size_t,
    ]
    lib.axon_start_nrt_profile.restype = ctypes.c_int64
    lib.axon_stop_nrt_profile.argtypes = [ctypes.c_char_p]
    lib.axon_stop_nrt_profile.restype = ctypes.c_int64

    @contextlib.contextmanager
    def _hook(output_dir, device_ids):
        import jax

        jax.devices()
        if device_ids:
            ids = (ctypes.c_int64 * len(device_ids))(*device_ids)
            rc = lib.axon_start_nrt_profile(ids, len(device_ids))
        else:
            rc = lib.axon_start_nrt_profile(None, 0)
        if rc != 0:
            raise RuntimeError(f"axon_start_nrt_profile rc={rc}")
        try:
            yield
        finally:
            n = lib.axon_stop_nrt_profile(str(output_dir).encode())
            print(f"profile: {n} file(s) written to {output_dir}", file=sys.stderr)

    mod = types.ModuleType("antenv.axon_hooks")
    mod.get_axon_ntff_profile_hook = lambda: _hook
    mod.set_axon_ntff_profile_hook = lambda h: None
    sys.modules["antenv.axon_hooks"] = mod

    import concourse.bass_utils as bu

    bu.upload_artifacts = lambda tmpdir: "local://" + str(tmpdir)


# revision 11
# speedup vs baseline: 1.9346x; 1.1771x over previous
"""Trainium2 Bass kernel for nn_NodeBlock (gnn_message_passing).

reference semantics:
    agg_mesh  = segment_sum(edge_attr, receivers, N)
    agg_world = segment_sum(edge_world_attr, receivers_world, N)
    h = concat([node_attr, agg_mesh, agg_world], -1)   # [N, 3D]
    h = relu(h @ W1 + b1) @ W2 + b2
    out = layernorm(h) * gamma + beta

Strategy (8 cores, nodes sharded by owner, edges partitioned by receiver
owner per the graph-partitioning hint):
  - 256-node supertiles; edges sorted by receiver and packed on host into
    per-supertile 128-row chunks (partition-major layout so DMA moves
    multi-KB contiguous runs per partition).
  - scatter = sum over chunks of edge_chunk^T @ P, accumulated transposed
    [D, node] in PSUM. Edges ship as bf16 hi + bf16 lo (exact to ~1e-5;
    2x faster PE + FWL weight loads vs fp32). P is one-hot built once per
    (supertile, stream) with a single broadcast tensor_tensor is_equal,
    narrowed to a per-chunk node window [base_c, base_c+W_c) shared across
    cores (sorted edges span few nodes per chunk). A zeroing matmul
    initializes the full PSUM tile so window gaps stay zero.
  - MLP in fp32: y^T = sum_j W1_j^T h_j^T directly from the transposed
    aggregates (node_attr arrives pre-transposed); z via lhsT=relu(y^T)
    against W2 augmented with a row-sum column so the LayerNorm mean falls
    out of the matmul for free; LN along the free dim with ACT-side
    square/accumulate and a fused tensor_scalar epilogue.
"""

import os

import numpy as np

LN_EPS = 1e-5
NC_CORES = 8
P = 128
SUP = 256  # nodes per supertile


def _build_program(cfg):
    import concourse.bass as bass
    import concourse.bacc as bacc
    import concourse.tile as tile
    from concourse import mybir

    f32 = mybir.dt.float32
    bf16 = mybir.dt.bfloat16
    TS = cfg["TS"]
    D = cfg["D"]
    NPAD = TS * SUP
    CHm, CHw = cfg["CHm"], cfg["CHw"]          # per-supertile chunk counts
    offm, offw = cfg["offm"], cfg["offw"]      # col offsets (in chunks)
    basem, basew = cfg["basem"], cfg["basew"]  # [s][c] window base
    wm, ww = cfg["wm"], cfg["ww"]              # [s][c] window width
    Wmax = cfg["Wmax"]
    TCm, TCw = offm[-1], offw[-1]              # total chunks per stream
    triv_affine = cfg["triv_affine"]
    b2mean = cfg["b2mean"]

    f32r = mybir.dt.float32r if cfg["use_f32r"] else mybir.dt.float32
    fp8 = mybir.dt.bfloat16 if cfg['bf16_lo'] else mybir.dt.float8e5
    nc = bacc.Bacc("TRN2")

    emh = nc.dram_tensor("emh", [P, TCm * P], bf16, kind="ExternalInput")
    eml = nc.dram_tensor("eml", [P, TCm * P], fp8, kind="ExternalInput")
    ewh = nc.dram_tensor("ewh", [P, TCw * P], bf16, kind="ExternalInput")
    ewl = nc.dram_tensor("ewl", [P, TCw * P], fp8, kind="ExternalInput")
    rm = nc.dram_tensor("rm", [P, TCm], bf16, kind="ExternalInput")
    rw = nc.dram_tensor("rw", [P, TCw], bf16, kind="ExternalInput")
    ndT = nc.dram_tensor("ndT", [P, NPAD], f32r, kind="ExternalInput")
    w1 = nc.dram_tensor("w1", [3 * P, D], f32r, kind="ExternalInput")
    w2e = nc.dram_tensor("w2e", [P, D + 2], f32r, kind="ExternalInput")
    b1 = nc.dram_tensor("b1", [P, 1], f32, kind="ExternalInput")
    iot = nc.dram_tensor("iot", [P, Wmax], bf16, kind="ExternalInput")
    zer = nc.dram_tensor("zer", [P, 2 * SUP], bf16, kind="ExternalInput")
    if not triv_affine:
        gb = nc.dram_tensor("gb", [P, D], f32, kind="ExternalInput")
        bb = nc.dram_tensor("bb", [P, D], f32, kind="ExternalInput")
        b2b = nc.dram_tensor("b2b", [P, D], f32, kind="ExternalInput")
    outd = nc.dram_tensor("out", [NPAD, D], f32, kind="ExternalOutput")

    with tile.TileContext(nc) as tc:
        with (
            tc.tile_pool(name="consts", bufs=1) as consts,
            tc.tile_pool(name="edges", bufs=4) as edges,
            tc.tile_pool(name="ponehot", bufs=3) as ponehot,
            tc.tile_pool(name="aggs", bufs=3) as aggs,
            tc.tile_pool(name="work", bufs=3) as work,
            tc.tile_pool(name="small", bufs=6) as small,
            tc.tile_pool(name="psag", bufs=3, space="PSUM") as psag,
            tc.tile_pool(name="psy", bufs=2, space="PSUM") as psy,
            tc.tile_pool(name="psz", bufs=2, space="PSUM") as psz,
        ):
            w1s = consts.tile([P, 3, D], f32r)
            nc.sync.dma_start(out=w1s, in_=w1[:, :].rearrange("(j p) d -> p j d", p=P))
            w2s = consts.tile([P, D + 2], f32r)
            nc.sync.dma_start(out=w2s, in_=w2e[:, :])
            b1s = consts.tile([P, 1], f32)
            nc.sync.dma_start(out=b1s, in_=b1[:, :])
            iots = consts.tile([P, Wmax], bf16)
            nc.sync.dma_start(out=iots, in_=iot[:, :])
            zers = consts.tile([P, 2 * SUP], bf16)
            nc.sync.dma_start(out=zers, in_=zer[:, :])
            epss = consts.tile([P, 1], f32)
            nc.vector.memset(epss, LN_EPS)
            ndTs = consts.tile([P, NPAD], f32r)
            nc.sync.dma_start(out=ndTs, in_=ndT[:, :])
            rms = consts.tile([P, TCm], bf16)
            nc.sync.dma_start(out=rms, in_=rm[:, :])
            rws = consts.tile([P, TCw], bf16)
            nc.sync.dma_start(out=rws, in_=rw[:, :])
            if not triv_affine:
                gbs = consts.tile([P, D], f32)
                nc.sync.dma_start(out=gbs, in_=gb[:, :])
                bbs = consts.tile([P, D], f32)
                nc.sync.dma_start(out=bbs, in_=bb[:, :])
                b2s = consts.tile([P, D], f32)
                nc.sync.dma_start(out=b2s, in_=b2b[:, :])

            def onehot(r_all, off_s, ch_s, tag):
                """P[e, c, n] = (iota[n] == r[e, c]) in one broadcast TT."""
                pm = ponehot.tile([P, ch_s, Wmax], bf16, tag=tag)
                r_sl = r_all[:, off_s : off_s + ch_s]
                r_b = bass.AP(
                    tensor=r_sl.tensor,
                    offset=r_sl.offset,
                    ap=[r_sl.ap[0], r_sl.ap[1], [0, Wmax]],
                )
                i_sl = iots[:, :]
                i_b = bass.AP(
                    tensor=i_sl.tensor,
                    offset=i_sl.offset,
                    ap=[i_sl.ap[0], [0, ch_s], i_sl.ap[1]],
                )
                nc.vector.tensor_tensor(
                    out=pm, in0=r_b, in1=i_b, op=mybir.AluOpType.is_equal
                )
                return pm

            def scatter(eh_t, el_t, pm, ps, col0, ch_s, base_s, w_s, last):
                for c in range(ch_s):
                    for op_t in (eh_t, el_t):
                        b = col0 + base_s[c]
                        nc.tensor.matmul(
                            out=ps[:, b : b + w_s[c]],
                            lhsT=op_t[:, c * P : (c + 1) * P],
                            rhs=pm[:, c, 0 : w_s[c]],
                            start=False,
                            stop=last and (c == ch_s - 1) and (op_t is el_t),
                            skip_group_check=True,
                        )

            for s in range(TS):
                chm, chw = CHm[s], CHw[s]
                emh_t = edges.tile([P, chm * P], bf16, tag="emh")
                nc.sync.dma_start(
                    out=emh_t, in_=emh[:, offm[s] * P : (offm[s] + chm) * P]
                )
                eml_t = edges.tile([P, chm * P], fp8, tag="eml")
                nc.scalar.dma_start(
                    out=eml_t, in_=eml[:, offm[s] * P : (offm[s] + chm) * P]
                )
                ewh_t = edges.tile([P, chw * P], bf16, tag="ewh")
                nc.sync.dma_start(
                    out=ewh_t, in_=ewh[:, offw[s] * P : (offw[s] + chw) * P]
                )
                ewl_t = edges.tile([P, chw * P], fp8, tag="ewl")
                nc.scalar.dma_start(
                    out=ewl_t, in_=ewl[:, offw[s] * P : (offw[s] + chw) * P]
                )

                pm = onehot(rms, offm[s], chm, "pm")
                pw = onehot(rws, offw[s], chw, "pw")

                agg_ps = psag.tile([P, 2 * SUP], f32, tag="agg")
                nc.tensor.matmul(
                    out=agg_ps,
                    lhsT=zers[:, :P],
                    rhs=zers[:, :],
                    start=True,
                    stop=False,
                    skip_group_check=True,
                )
                scatter(emh_t, eml_t, pm, agg_ps, 0, chm, basem[s], wm[s], False)
                scatter(ewh_t, ewl_t, pw, agg_ps, SUP, chw, basew[s], ww[s], True)

                aggsb = aggs.tile([P, 2 * SUP], f32r, tag="aggsb")
                nc.any.tensor_copy(out=aggsb, in_=agg_ps)

                # MLP: y^T[dout, n] = sum_k W1[k, dout] h^T[k, n]
                y_ps = psy.tile([P, SUP], f32, tag="y")
                nc.tensor.matmul(
                    out=y_ps,
                    lhsT=w1s[:, 0, :],
                    rhs=ndTs[:, s * SUP : (s + 1) * SUP],
                    start=True,
                    stop=False,
                )
                nc.tensor.matmul(
                    out=y_ps,
                    lhsT=w1s[:, 1, :],
                    rhs=aggsb[:, 0:SUP],
                    start=False,
                    stop=False,
                )
                nc.tensor.matmul(
                    out=y_ps, lhsT=w1s[:, 2, :], rhs=aggsb[:, SUP : 2 * SUP], start=False, stop=True
                )
                yr = work.tile([P, SUP], f32r, tag="yr")
                nc.scalar.activation(
                    out=yr,
                    in_=y_ps,
                    func=mybir.ActivationFunctionType.Relu,
                    bias=b1s[:, :],
                    scale=1.0,
                )
                # z_ext[n, :D] = z, z_ext[n, D] = sum_dz z  (for the mean)
                z_ps = psz.tile([P, 2, D + 2], f32, tag="z")
                for h in range(2):
                    nc.tensor.matmul(
                        out=z_ps[:, h, :],
                        lhsT=yr[:, h * P : (h + 1) * P],
                        rhs=w2s,
                        start=True,
                        stop=True,
                    )
                for h in range(2):
                    zh = z_ps[:, h, 0:D]
                    negmu = small.tile([P, 1], f32, tag="negmu")
                    nc.scalar.activation(
                        out=negmu,
                        in_=z_ps[:, h, D : D + 1],
                        func=mybir.ActivationFunctionType.Copy,
                        bias=-b2mean,
                        scale=-1.0 / D,
                    )
                    sq = work.tile([P, D], f32, tag="sq")
                    sqsum = small.tile([P, 1], f32, tag="sqsum")
                    if triv_affine:
                        zc = zh  # centered input is z + negmu via ACT bias
                        nc.scalar.activation(
                            out=sq,
                            in_=zh,
                            func=mybir.ActivationFunctionType.Square,
                            bias=negmu[:, :],
                            scale=1.0,
                            accum_out=sqsum,
                        )
                    else:
                        # t0 = z + b2 - mu (materialized; feeds var and out)
                        t0 = work.tile([P, D], f32, tag="t0")
                        nc.vector.scalar_tensor_tensor(
                            out=t0,
                            in0=zh,
                            scalar=negmu,
                            in1=b2s,
                            op0=mybir.AluOpType.add,
                            op1=mybir.AluOpType.add,
                        )
                        nc.scalar.activation(
                            out=sq,
                            in_=t0,
                            func=mybir.ActivationFunctionType.Square,
                            accum_out=sqsum,
                        )
                    std = small.tile([P, 1], f32, tag="std")
                    nc.scalar.activation(
                        out=std,
                        in_=sqsum,
                        func=mybir.ActivationFunctionType.Sqrt,
                        bias=epss[:, :],
                        scale=1.0 / D,
                    )
                    rstd = small.tile([P, 1], f32, tag="rstd")
                    nc.vector.reciprocal(out=rstd, in_=std)
                    outt = work.tile([P, D], f32, tag="outt")
                    if triv_affine:
                        nc.vector.tensor_scalar(
                            out=outt,
                            in0=zh,
                            scalar1=negmu,
                            scalar2=rstd,
                            op0=mybir.AluOpType.add,
                            op1=mybir.AluOpType.mult,
                        )
                    else:
                        # (z + b2 - mu) * rstd * gamma + beta
                        t1 = work.tile([P, D], f32, tag="t1")
                        nc.vector.scalar_tensor_tensor(
                            out=t1,
                            in0=t0,
                            scalar=rstd,
                            in1=gbs,
                            op0=mybir.AluOpType.mult,
                            op1=mybir.AluOpType.mult,
                        )
                        nc.vector.tensor_tensor(
                            out=outt, in0=t1, in1=bbs, op=mybir.AluOpType.add
                        )
                    row = s * SUP + h * P
                    nc.gpsimd.dma_start(out=outd[row : row + P, :], in_=outt)

    nc.finalize()
    return nc


def _pack_edges(edge_attr, recv, NPC, TS, n_cores, bf16, fp8):
    """Sort by receiver, partition by owner core, pack into per-supertile
    128-row chunks in partition-major bf16 hi/lo layout with shared
    per-chunk node windows."""
    D = edge_attr.shape[1]
    order = np.argsort(recv, kind="stable")
    r_sorted = recv[order]
    c_ids = r_sorted // NPC
    loc = r_sorted - c_ids * NPC
    s_ids = loc // SUP
    rl = loc - s_ids * SUP
    g_ids = c_ids * TS + s_ids
    cnt = np.bincount(g_ids, minlength=n_cores * TS)
    starts = np.cumsum(cnt) - cnt
    pos = np.arange(len(r_sorted)) - starts[g_ids]
    ch_ids = pos // P

    # per-supertile chunk count = max over cores
    CH = np.maximum(
        1, np.ceil(cnt.reshape(n_cores, TS) / P).astype(np.int64).max(axis=0)
    )  # [TS]
    off = np.concatenate([[0], np.cumsum(CH)]).astype(np.int64)  # [TS+1]
    TC = int(off[-1])

    # shared window base/width per (supertile, chunk)
    CHmax = int(CH.max())
    minrl = np.full((n_cores * TS, CHmax), 10**9, np.int64)
    maxrl = np.full((n_cores * TS, CHmax), -1, np.int64)
    idx = g_ids * CHmax + ch_ids
    np.minimum.at(minrl.reshape(-1), idx, rl)
    np.maximum.at(maxrl.reshape(-1), idx, rl)
    minrl = minrl.reshape(n_cores, TS, CHmax).min(axis=0)  # [TS, CHmax]
    maxrl = maxrl.reshape(n_cores, TS, CHmax).max(axis=0)
    base = np.clip(minrl, 0, SUP)
    width = np.clip(maxrl - base + 1, 1, SUP)
    base_l = [[int(base[s, c]) for c in range(CH[s])] for s in range(TS)]
    w_l = [[int(width[s, c]) for c in range(CH[s])] for s in range(TS)]
    Wmax = max(1, int(max(max(w) for w in w_l)))
    assert Wmax <= 200

    # destination column in the [P, TC*P] partition-major buffer:
    # supertile s, chunk c, edge e (partition), dim d -> [e, (off[s]+c)*P + d]
    part = pos - ch_ids * P  # partition = edge index within chunk
    colchunk = off[s_ids] + ch_ids  # global chunk column
    ea = edge_attr[order].astype(np.float32)
    hi = ea.astype(bf16)
    lo = (ea - hi.astype(np.float32)).astype(fp8)

    emh = np.zeros((n_cores, P, TC * P), bf16)
    eml = np.zeros((n_cores, P, TC * P), fp8)
    rlp = np.full((n_cores, P, TC), 250.0, bf16)  # sentinel: never matches
    cols = (colchunk * P)[:, None] + np.arange(D)[None, :]
    emh[c_ids[:, None], part[:, None], cols] = hi
    eml[c_ids[:, None], part[:, None], cols] = lo
    rl_rel = rl - base[s_ids, ch_ids]
    rlp[c_ids, part, colchunk] = rl_rel.astype(bf16)

    return emh, eml, rlp, [int(x) for x in CH], [int(x) for x in off], base_l, w_l, Wmax


def kernel(**inputs):
    import ml_dtypes
    from concourse.bass_utils import run_bass_kernel_spmd

    bf16 = np.dtype(ml_dtypes.bfloat16)
    fp8 = (
        np.dtype(ml_dtypes.bfloat16)
        if os.environ.get("KERNEL_BF16_LO", "") != ""
        else np.dtype(ml_dtypes.float8_e5m2)
    )

    node_attr = np.asarray(inputs["node_attr"], np.float32)
    edge_attr = np.asarray(inputs["edge_attr"], np.float32)
    edge_world_attr = np.asarray(inputs["edge_world_attr"], np.float32)
    recv = np.asarray(inputs["receivers"]).astype(np.int64)
    recv_w = np.asarray(inputs["receivers_world"]).astype(np.int64)
    W1 = np.asarray(inputs["W1"], np.float32)
    b1 = np.asarray(inputs["b1"], np.float32)
    W2 = np.asarray(inputs["W2"], np.float32)
    b2 = np.asarray(inputs["b2"], np.float32)
    gamma = np.asarray(inputs["gamma"], np.float32)
    beta = np.asarray(inputs["beta"], np.float32)

    N, D = node_attr.shape
    assert D == P and N % NC_CORES == 0
    NPC = N // NC_CORES
    TS = (NPC + SUP - 1) // SUP
    NPAD = TS * SUP

    emh, eml, rmp, CHm, offm, basem, wm, Wm = _pack_edges(
        edge_attr, recv, NPC, TS, NC_CORES, bf16, fp8
    )
    ewh, ewl, rwp, CHw, offw, basew, ww, Ww = _pack_edges(
        edge_world_attr, recv_w, NPC, TS, NC_CORES, bf16, fp8
    )
    Wmax = max(Wm, Ww)

    ndT = np.zeros((NC_CORES, P, NPAD), np.float32)
    for c in range(NC_CORES):
        ndT[c, :, :NPC] = node_attr[c * NPC : (c + 1) * NPC].T

    triv_affine = (
        not b2.any() and not beta.any() and bool(np.all(gamma == 1.0))
    )
    cfg = {
        "TS": TS,
        "D": D,
        "CHm": CHm,
        "CHw": CHw,
        "offm": offm,
        "offw": offw,
        "basem": basem,
        "basew": basew,
        "wm": wm,
        "ww": ww,
        "Wmax": Wmax,
        "triv_affine": triv_affine,
        "use_f32r": os.environ.get("KERNEL_F32R", "") != "",
        "bf16_lo": os.environ.get("KERNEL_BF16_LO", "") != "",
        "b2mean": float(b2.mean()),
    }
    nc = _build_program(cfg)

    iota = np.tile(np.arange(Wmax, dtype=np.float32), (P, 1)).astype(bf16)
    zeros_b = np.zeros((P, 2 * SUP), bf16)
    w2e = np.concatenate(
        [W2, W2.sum(axis=1, keepdims=True), np.zeros((P, 1), np.float32)], axis=1
    ).astype(np.float32)
    b1c = np.ascontiguousarray(b1.reshape(P, 1))

    in_maps = []
    for c in range(NC_CORES):
        m = {
            "emh": emh[c],
            "eml": eml[c],
            "ewh": ewh[c],
            "ewl": ewl[c],
            "rm": rmp[c],
            "rw": rwp[c],
            "ndT": ndT[c],
            "w1": W1,
            "w2e": w2e,
            "b1": b1c,
            "iot": iota,
            "zer": zeros_b,
        }
        if not triv_affine:
            m["gb"] = np.tile(gamma, (P, 1)).astype(np.float32)
            m["bb"] = np.tile(beta, (P, 1)).astype(np.float32)
            m["b2b"] = np.tile(b2, (P, 1)).astype(np.float32)
        in_maps.append(m)

    prof_dir = os.environ.get("KERNEL_PROFILE_DIR")
    trace = False
    if prof_dir:
        try:
            _install_profile_hook()
            trace = True
        except Exception as e:  # profiling is best-effort
            print(f"profile hook unavailable: {e}")

    res = run_bass_kernel_spmd(
        nc,
        in_maps,
        core_ids=list(range(NC_CORES)),
        trace=trace,
        tmpdir=prof_dir if trace else None,
    )
    if trace:
        print(f"HW exec time: {res.exec_time_ns} ns")

    out = np.concatenate(
        [res.results[c]["out"][:NPC] for c in range(NC_CORES)], axis=0
    )
    return out


def _install_profile_hook():
    """Register the axon NTFF profile hook (the boot path skips it when
    antenv.axon_hooks is absent) and neuter the artifact upload."""
    import contextlib
    import ctypes
    import sys
    import types

    lib = ctypes.CDLL("/opt/axon/libaxon_pjrt.so")
    lib.axon_start_nrt_profile.argtypes = [
        ctypes.POINTER(ctypes.c_int64),
        ctypes.c_size_t,
    ]
    lib.axon_start_nrt_profile.restype = ctypes.c_int64
    lib.axon_stop_nrt_profile.argtypes = [ctypes.c_char_p]
    lib.axon_stop_nrt_profile.restype = ctypes.c_int64

    @contextlib.contextmanager
    def _hook(output_dir, device_ids):
        import jax

        jax.devices()
        if device_ids:
            ids = (ctypes.c_int64 * len(device_ids))(*device_ids)
            rc = lib.axon_start_nrt_profile(ids, len(device_ids))
        else:
            rc = lib.axon_start_nrt_profile(None, 0)
        if rc != 0:
            raise RuntimeError(f"axon_start_nrt_profile rc={rc}")
        try:
            yield
        finally:
            n = lib.axon_stop_nrt_profile(str(output_dir).encode())
            print(f"profile: {n} file(s) written to {output_dir}", file=sys.stderr)

    mod = types.ModuleType("antenv.axon_hooks")
    mod.get_axon_ntff_profile_hook = lambda: _hook
    mod.set_axon_ntff_profile_hook = lambda h: None
    sys.modules["antenv.axon_hooks"] = mod

    import concourse.bass_utils as bu

    bu.upload_artifacts = lambda tmpdir: "local://" + str(tmpdir)
